# revision 21
# baseline (speedup 1.0000x reference)
"""Dense causal MHA (B=2, S=2048, H=16, D=128, hidden=2048) on 8 Trainium2 cores.

Sharding: data-parallel over batch (2) x tensor-parallel over head groups
(4 heads/core).  Core c handles batch c//4, heads 4*(c%4) .. 4*(c%4)+3.
Each core computes a partial output (its heads' contribution to the out
projection, with bo/4 folded in); the host sums the 4 partials per batch.

v3 layout (everything bf16 except PSUM accumulation, reciprocals and the
final output, which stay f32):
  - single pass over x: phase 1 computes K^T (roped), V and Q^T (roped) for
    the whole sequence, time-multiplexing PSUM banks K->Q within a chunk.
  - phase 2 is attention + out-projection only, software-pipelined with a
    3-tile lookahead so the PE never waits on the exp chain.
  - psum banks phase 2: shared(psd+pso) 2 + context 2 + scores/psb 4, with
    pool-creation order chosen so the first-touched banks are the ones the
    last phase-1 chunk's Act engine drains earliest (V banks).
  - softmax denominator via ones-vector matmul into the shared pool; the
    1/sum row is broadcast across partitions by a [1,128]-ones matmul
    (f32r fast path) deferred one head so the PE never waits on it.
  - out-projection deferred one chunk so its matmuls fill the PE while the
    next chunk's attention warms up.
"""

import sys

sys.path.insert(0, "/opt/trn_rl_repo")

from contextlib import ExitStack

import numpy as np
import ml_dtypes

import concourse.tile as tile
from concourse import bacc, mybir
from concourse.bass_utils import run_bass_kernel_spmd

S = 2048
HID = 2048
D = 128
LH = 4            # heads per core
DL = LH * D       # 512 local inner dims
SC = 512          # chunk size (q and kv)
NSC = S // SC     # 4
HCH = HID // 128  # 16 contraction chunks
N_CORES = 8

f32 = mybir.dt.float32
f32r = mybir.dt.float32r
bf16 = mybir.dt.bfloat16
Exp = mybir.ActivationFunctionType.Exp
Ident = mybir.ActivationFunctionType.Identity

_CACHE = {}


def _build_nc():
    nc = bacc.Bacc("TRN2", target_bir_lowering=False, debug=False,
                   num_devices=N_CORES)

    def din(name, shape, dt=bf16):
        return nc.dram_tensor(name, shape, dt, kind="ExternalInput").ap()

    xT = din("xT", [HID, S])
    wqT = din("wqT", [HID, DL])
    wkT = din("wkT", [HID, DL])
    wvT = din("wvT", [HID, DL])
    woT = din("woT", [DL, HID])
    bq2 = din("bq2", [128, LH], f32)
    bk2 = din("bk2", [128, LH], f32)
    cosT = din("cosT", [128, S])
    sinT = din("sinT", [128, S])
    trim = din("trim", [128, 128])
    onec = din("onec", [128, 1])
    oner = din("oner", [1, 128], f32r)
    out = nc.dram_tensor("out", [S, HID], bf16, kind="ExternalOutput").ap()

    with tile.TileContext(nc) as tc, ExitStack() as ctx:
        P = ctx.enter_context(tc.tile_pool(name="persist", bufs=1))
        WQP = ctx.enter_context(tc.tile_pool(name="wq", bufs=1))
        WOP = ctx.enter_context(tc.tile_pool(name="wo", bufs=1))

        K_sb = [P.tile([128, S], bf16, tag=f"K{d}", name=f"Ksb{d}")
                for d in range(LH)]
        Q_sb = [P.tile([128, S], bf16, tag=f"Q{d}", name=f"Qsb{d}")
                for d in range(LH)]
        V_sb = [P.tile([128, DL], bf16, tag=f"V{t}", name=f"Vsb{t}")
                for t in range(S // 128)]
        cos_sb = P.tile([128, S], bf16, tag="cos")
        sin_sb = P.tile([128, S], bf16, tag="sin")
        tri_sb = P.tile([128, 128], bf16, tag="tri")
        bq_sb = P.tile([128, LH], f32, tag="bq")
        bk_sb = P.tile([128, LH], f32, tag="bk")
        onec_sb = P.tile([128, 1], bf16, tag="onec")
        oner_sb = P.tile([1, 128], f32r, tag="oner")
        wo_sb = [WOP.tile([128, HID], bf16, tag=f"wo{h}", name=f"wosb{h}")
                 for h in range(LH)]

        def rope(pool, raw, dst, sl):
            """dst = raw*cos + rotate_half(raw)*sin; the rotate is folded
            into partition-offset reads against a half-sign-flipped sin
            table (sin_sb rows >=64 carry the minus sign)."""
            m1 = pool.tile([128, SC], bf16, tag="rm1", bufs=2)
            nc.vector.tensor_mul(m1[:], raw[:], cos_sb[:, sl])
            m2 = pool.tile([128, SC], bf16, tag="rm2", bufs=2)
            nc.vector.tensor_mul(m2[0:64, :], raw[64:128, :],
                                 sin_sb[64:128, sl])
            nc.vector.tensor_mul(m2[64:128, :], raw[0:64, :],
                                 sin_sb[0:64, sl])
            nc.vector.tensor_add(dst, m1[:], m2[:])

        # ---- phase 1: K^T (roped), V, Q^T (roped) for the whole sequence ----
        with tc.tile_pool(name="p1w", bufs=1) as WP, \
             tc.tile_pool(name="p1x", bufs=1) as XP, \
             tc.tile_pool(name="p1t", bufs=2) as TP, \
             tc.tile_pool(name="p1kq", bufs=4, space="PSUM") as PKQ, \
             tc.tile_pool(name="p1v", bufs=4, space="PSUM") as PV:
            wk_sb = [WP.tile([128, DL], bf16, tag=f"wk{h}", name=f"wksb{h}")
                     for h in range(HCH)]
            wv_sb = [WP.tile([128, DL], bf16, tag=f"wv{h}", name=f"wvsb{h}")
                     for h in range(HCH)]
            wq_sb = [WQP.tile([128, DL], bf16, tag=f"wq{h}", name=f"wqsb{h}")
                     for h in range(HCH)]
            # one chunk of x tiles in flight plus the next being fetched
            x_t = [[XP.tile([128, SC], bf16, tag=f"x{h}_{j % 2}",
                            name=f"xt{j}_{h}") for h in range(HCH)]
                   for j in range(NSC)]

            # two HWDGE queues in parallel: weights stream on the SP queue
            # while x chunk 0 streams on the Activation queue, so chunk-0
            # K-projection (one wk+x pair per 864ns) is never DMA-starved.
            for h in range(HCH):
                nc.sync.dma_start(wk_sb[h][:], wkT[128 * h:128 * (h + 1), :])
                nc.scalar.dma_start(x_t[0][h][:], xT[128 * h:128 * (h + 1),
                                                     0:SC])
            for h in range(HCH):
                nc.sync.dma_start(wv_sb[h][:], wvT[128 * h:128 * (h + 1), :])
            nc.scalar.dma_start(cos_sb[:], cosT[:])
            nc.scalar.dma_start(sin_sb[:], sinT[:])
            nc.scalar.dma_start(tri_sb[:], trim[:])
            nc.scalar.dma_start(onec_sb[:], onec[:])
            nc.scalar.dma_start(oner_sb[:], oner[:])
            nc.scalar.dma_start(bq_sb[:], bq2[:])
            nc.scalar.dma_start(bk_sb[:], bk2[:])
            # trigger the one-off ACT_TABLE_LOAD while the PE is projecting
            warm = TP.tile([128, 1], f32, tag="warm", bufs=1)
            nc.scalar.activation(warm[:], bk_sb[:, 0:1], Exp)
            for h in range(HCH):
                nc.sync.dma_start(wq_sb[h][:], wqT[128 * h:128 * (h + 1), :])
            for h in range(HCH):
                nc.scalar.dma_start(x_t[1][h][:], xT[128 * h:128 * (h + 1),
                                                     SC:2 * SC])
            for h in range(LH):
                nc.scalar.dma_start(wo_sb[h][:], woT[128 * h:128 * (h + 1), :])

            for j in range(NSC):
                sl = slice(SC * j, SC * (j + 1))
                if j >= 1:
                    nj = j + 1
                    if nj < NSC:
                        for h in range(HCH):
                            nc.sync.dma_start(
                                x_t[nj][h][:],
                                xT[128 * h:128 * (h + 1),
                                   SC * nj:SC * (nj + 1)])
                # K projection (4 banks), then V (4 banks), then Q reusing
                # K's banks after the Act engine drained them.
                psk = [PKQ.tile([128, SC], f32, tag="pkq",
                                name=f"psk{j}_{d}") for d in range(LH)]
                for h in range(HCH):
                    for d in range(LH):
                        nc.tensor.matmul(psk[d][:],
                                         wk_sb[h][:, 128 * d:128 * (d + 1)],
                                         x_t[j][h][:], start=(h == 0),
                                         stop=(h == HCH - 1))
                psv = [PV.tile([128, DL], f32, tag="pv",
                               name=f"psv{j}_{st}") for st in range(4)]
                for h in range(HCH):
                    for st in range(4):
                        nc.tensor.matmul(psv[st][:],
                                         x_t[j][h][:, 128 * st:128 * (st + 1)],
                                         wv_sb[h][:], start=(h == 0),
                                         stop=(h == HCH - 1))
                for d in range(LH):
                    kraw = TP.tile([128, SC], bf16, tag="kraw", bufs=2)
                    nc.scalar.activation(kraw[:], psk[d][:], Ident,
                                         bias=bk_sb[:, d:d + 1], scale=1.0)
                    rope(TP, kraw, K_sb[d][:, sl], sl)
                psq = [PKQ.tile([128, SC], f32, tag="pkq",
                                name=f"psq{j}_{d}") for d in range(LH)]
                for h in range(HCH):
                    for d in range(LH):
                        nc.tensor.matmul(psq[d][:],
                                         wq_sb[h][:, 128 * d:128 * (d + 1)],
                                         x_t[j][h][:], start=(h == 0),
                                         stop=(h == HCH - 1))
                for st in range(4):
                    nc.scalar.copy(V_sb[4 * j + st][:], psv[st][:])
                for d in range(LH):
                    qraw = TP.tile([128, SC], bf16, tag="qraw", bufs=2)
                    nc.scalar.activation(qraw[:], psq[d][:], Ident,
                                         bias=bq_sb[:, d:d + 1], scale=1.0)
                    rope(TP, qraw, Q_sb[d][:, sl], sl)

        # ---- phase 2: attention (pipelined) + deferred out-projection ----
        # pool creation order fixes bank placement: SH+PC on the psq banks
        # (drained right after phase 1), PS on the psv banks (drained early).
        with tc.tile_pool(name="p2sh", bufs=2, space="PSUM") as SH, \
             tc.tile_pool(name="p2pc", bufs=2, space="PSUM") as PC, \
             tc.tile_pool(name="p2ps", bufs=4, space="PSUM") as PS, \
             tc.tile_pool(name="p2ex", bufs=6) as EX, \
             tc.tile_pool(name="p2rc", bufs=2) as RC, \
             tc.tile_pool(name="p2rb", bufs=2) as RB, \
             tc.tile_pool(name="p2ct", bufs=2) as CT, \
             tc.tile_pool(name="p2ot", bufs=4) as OT:

            def emit_outproj(j, ct, last=False):
                for oc in range(4):
                    osl = slice(SC * oc, SC * (oc + 1))
                    for qt in range(4):
                        # the PS (score) banks are idle during the deferred
                        # out-projection; rotating 4 of them keeps the pso
                        # chains well ahead of their copies.
                        pso = PS.tile([128, SC], f32, tag="ps",
                                      name=f"pso{j}_{oc}_{qt}")
                        for it in range(LH):
                            nc.tensor.matmul(
                                pso[:], ct[it][:, 128 * qt:128 * (qt + 1)],
                                wo_sb[it][:, osl], start=(it == 0),
                                stop=(it == LH - 1))
                        ot = OT.tile([128, SC], bf16, tag="ot",
                                     name=f"ot{j}_{oc}_{qt}")
                        if qt % 2 == 0:
                            nc.scalar.copy(ot[:], pso[:])
                        else:
                            nc.vector.tensor_copy(ot[:], pso[:])
                        # the final chunk's stores split across both HWDGE
                        # queues so the drain after the last matmul halves
                        dma_eng = nc.scalar if (last and qt % 2) else nc.sync
                        dma_eng.dma_start(
                            out[SC * j + 128 * qt:SC * j + 128 * (qt + 1),
                                osl], ot[:])

            pending = None   # (j, ct) outproj deferred one chunk
            ct = [None] * LH

            def emit_norm_tail(j, h, psc, den):
                """den broadcast + reciprocal + ct mul for head h of chunk j
                (deferred one head so the PE never waits on the den copy)."""
                psb = SH.tile([128, SC], f32, tag="sh", name=f"psb{j}_{h}")
                nc.tensor.matmul(psb[:], oner_sb[:], den[:],
                                 start=True, stop=True)
                rb = RB.tile([128, SC], f32, tag="rb", name=f"rb{j}_{h}")
                nc.vector.reciprocal_approx_fast(out=rb[:], in_=psb[:])
                cth = CT.tile([128, SC], bf16, tag=f"ct{h}",
                              name=f"ct{j}_{h}")
                nc.vector.tensor_mul(cth[:], psc[:], rb[:])
                ct[h] = cth

            for j in range(NSC):
                T = 4 * j + 4
                norm_pend = None

                for h in range(LH):
                    psc = PC.tile([128, SC], f32, tag="pc",
                                  name=f"psc{j}_{h}")
                    psd = SH.tile([1, SC], f32, tag="sh",
                                  name=f"psd{j}_{h}")
                    exs = [None] * T

                    def emit_score(t):
                        p = t - 4 * j  # >=0 for diagonal tiles
                        c0 = 128 * p if p > 0 else 0
                        cs = slice(c0, SC)
                        ps = PS.tile([128, SC], f32, tag="ps",
                                     name=f"pss{j}_{h}_{t}")
                        nc.tensor.matmul(ps[:, cs],
                                         K_sb[h][:, 128 * t:128 * (t + 1)],
                                         Q_sb[h][:, SC * j + c0:SC * (j + 1)],
                                         start=True, stop=True)
                        ex = EX.tile([128, SC], bf16, tag="ex",
                                     name=f"ex{j}_{h}_{t}")
                        nc.scalar.activation(ex[:, cs], ps[:, cs], Exp)
                        if p >= 0:
                            dsl = slice(128 * p, 128 * (p + 1))
                            nc.vector.tensor_mul(ex[:, dsl], ex[:, dsl],
                                                 tri_sb[:])
                        exs[t] = ex

                    for t in range(min(3, T)):
                        emit_score(t)
                    for t in range(T):
                        if t + 3 < T:
                            emit_score(t + 3)
                        p = t - 4 * j
                        cs = slice(128 * p if p > 0 else 0, SC)
                        nc.tensor.matmul(psd[:, cs], onec_sb[:],
                                         exs[t][:, cs],
                                         start=(t == 0), stop=(t == T - 1))
                        nc.tensor.matmul(psc[:, cs],
                                         V_sb[t][:, 128 * h:128 * (h + 1)],
                                         exs[t][:, cs], start=(t == 0),
                                         stop=(t == T - 1))
                    den = RC.tile([1, SC], f32r, tag="den",
                                  name=f"den{j}_{h}")
                    nc.vector.tensor_copy(den[:], psd[:])
                    if norm_pend is not None:
                        emit_norm_tail(*norm_pend)
                    norm_pend = (j, h, psc, den)
                if pending is not None:
                    emit_outproj(*pending)
                emit_norm_tail(*norm_pend)
                pending = (j, list(ct))
            emit_outproj(*pending, last=True)
    nc.compile()
    return nc


def _get_nc():
    if "nc" not in _CACHE:
        _CACHE["nc"] = _build_nc()
    return _CACHE["nc"]


def _consts():
    if "consts" not in _CACHE:
        inv = (10000.0 ** (-np.arange(0, D, 2, dtype=np.float64) / D))
        t = np.arange(S, dtype=np.float64)
        fr = np.outer(t, inv)                      # [S, 64]
        cos = np.concatenate([np.cos(fr)] * 2, 1).T
        sin = np.concatenate([np.sin(fr)] * 2, 1).T.copy()
        sin[64:] *= -1.0
        tri = (np.arange(128)[:, None] <= np.arange(128)[None, :])
        _CACHE["consts"] = {
            "cosT": np.ascontiguousarray(cos.astype(ml_dtypes.bfloat16)),
            "sinT": np.ascontiguousarray(sin.astype(ml_dtypes.bfloat16)),
            "trim": np.ascontiguousarray(
                tri.astype(ml_dtypes.bfloat16)),
            "onec": np.ones((128, 1), ml_dtypes.bfloat16),
            "oner": np.ones((1, 128), np.float32),
        }
    return _CACHE["consts"]


def _marshal(hidden_states, Wq, bq, Wk, bk, Wv, bv, Wo, bo):
    consts = _consts()
    scale = 1.0 / np.sqrt(D)
    xTs = [np.ascontiguousarray(
        hidden_states[b].T.astype(ml_dtypes.bfloat16)) for b in range(2)]
    in_maps = []
    for c in range(N_CORES):
        b, hg = c // 4, c % 4
        rows = slice(DL * hg, DL * (hg + 1))
        m = dict(consts)
        m["xT"] = xTs[b]
        m["wqT"] = np.ascontiguousarray(
            (Wq[rows] * scale).T.astype(ml_dtypes.bfloat16))
        m["wkT"] = np.ascontiguousarray(Wk[rows].T.astype(ml_dtypes.bfloat16))
        m["wvT"] = np.ascontiguousarray(Wv[rows].T.astype(ml_dtypes.bfloat16))
        m["woT"] = np.ascontiguousarray(
            Wo[:, rows].T.astype(ml_dtypes.bfloat16))
        m["bq2"] = np.ascontiguousarray(
            (bq[rows] * scale).reshape(LH, 128).T.astype(np.float32))
        m["bk2"] = np.ascontiguousarray(
            bk[rows].reshape(LH, 128).T.astype(np.float32))
        in_maps.append(m)
    return in_maps


def _gather(results, bias):
    out = np.empty((2, S, HID), np.float32)
    for b in range(2):
        acc = results[4 * b]["out"].astype(np.float32).copy()
        for g in range(1, 4):
            acc += results[4 * b + g]["out"]
        out[b] = acc + bias
    return out


def _run(inputs, **kw):
    nc = _get_nc()
    in_maps = _marshal(**{k: np.asarray(v) for k, v in inputs.items()})
    return run_bass_kernel_spmd(nc, in_maps, core_ids=list(range(N_CORES)),
                                **kw)


def _host_bias(inputs):
    Wo = np.asarray(inputs["Wo"], np.float64)
    bv = np.asarray(inputs["bv"], np.float64)
    bo = np.asarray(inputs["bo"], np.float64)
    return (bo + Wo @ bv).astype(np.float32)


def kernel(**inputs):
    res = _run(inputs)
    return _gather(res.results, _host_bias(inputs))


def kernel_traced(**inputs):
    """Like kernel() but with NTFF profiling; returns (output, results)."""
    import types

    try:
        import antenv.axon_hooks  # noqa: F401
    except ImportError:
        from trn_agent_boot.trn_boot import _ntff_profile_via_ctypes
        hook = _ntff_profile_via_ctypes("/opt/axon/libaxon_pjrt.so")
        mod = types.ModuleType("antenv.axon_hooks")
        mod.get_axon_ntff_profile_hook = lambda: hook
        mod.set_axon_ntff_profile_hook = lambda h: None
        sys.modules["antenv.axon_hooks"] = mod
    res = _run(inputs, trace=True)
    return _gather(res.results, _host_bias(inputs)), res


# revision 27
# speedup vs baseline: 1.0307x; 1.0307x over previous
"""Dense causal MHA (B=2, S=2048, H=16, D=128, hidden=2048) on 8 Trainium2 cores.

Sharding: data-parallel over batch (2) x tensor-parallel over head groups
(4 heads/core).  Core c handles batch c//4, heads 4*(c%4) .. 4*(c%4)+3.
Each core computes a partial output (its heads' contribution to the out
projection, with bo/4 folded in); the host sums the 4 partials per batch.

v3 layout (everything bf16 except PSUM accumulation, reciprocals and the
final output, which stay f32):
  - single pass over x: phase 1 computes K^T (roped), V and Q^T (roped) for
    the whole sequence, time-multiplexing PSUM banks K->Q within a chunk.
  - phase 2 is attention + out-projection only, software-pipelined with a
    3-tile lookahead so the PE never waits on the exp chain.
  - psum banks phase 2: shared(psd+pso) 2 + context 2 + scores/psb 4, with
    pool-creation order chosen so the first-touched banks are the ones the
    last phase-1 chunk's Act engine drains earliest (V banks).
  - softmax denominator via ones-vector matmul into the shared pool; the
    1/sum row is broadcast across partitions by a [1,128]-ones matmul
    (f32r fast path) deferred one head so the PE never waits on it.
  - out-projection deferred one chunk so its matmuls fill the PE while the
    next chunk's attention warms up.
"""

import sys

sys.path.insert(0, "/opt/trn_rl_repo")

from contextlib import ExitStack

import numpy as np
import ml_dtypes

import concourse.tile as tile
from concourse import bacc, mybir
from concourse.bass_utils import run_bass_kernel_spmd

S = 2048
HID = 2048
D = 128
LH = 4            # heads per core
DL = LH * D       # 512 local inner dims
SC = 512          # chunk size (q and kv)
NSC = S // SC     # 4
HCH = HID // 128  # 16 contraction chunks
N_CORES = 8

f32 = mybir.dt.float32
f32r = mybir.dt.float32r
bf16 = mybir.dt.bfloat16
Exp = mybir.ActivationFunctionType.Exp
Ident = mybir.ActivationFunctionType.Identity

_CACHE = {}


def _build_nc():
    nc = bacc.Bacc("TRN2", target_bir_lowering=False, debug=False,
                   num_devices=N_CORES)

    def din(name, shape, dt=bf16):
        return nc.dram_tensor(name, shape, dt, kind="ExternalInput").ap()

    xT = din("xT", [HID, S])
    wqT = din("wqT", [HID, DL])
    wkT = din("wkT", [HID, DL])
    wvT = din("wvT", [HID, DL])
    woT = din("woT", [DL, HID])
    bq2 = din("bq2", [128, LH], f32)
    bk2 = din("bk2", [128, LH], f32)
    cosT = din("cosT", [128, S])
    sinT = din("sinT", [128, S])
    trim = din("trim", [128, 128])
    onec = din("onec", [128, 1])
    oner = din("oner", [1, 128], f32r)
    out = nc.dram_tensor("out", [S, HID], bf16, kind="ExternalOutput").ap()

    with tile.TileContext(nc) as tc, ExitStack() as ctx:
        P = ctx.enter_context(tc.tile_pool(name="persist", bufs=1))
        WQP = ctx.enter_context(tc.tile_pool(name="wq", bufs=1))
        WOP = ctx.enter_context(tc.tile_pool(name="wo", bufs=1))

        K_sb = [P.tile([128, S], bf16, tag=f"K{d}", name=f"Ksb{d}")
                for d in range(LH)]
        Q_sb = [P.tile([128, S], bf16, tag=f"Q{d}", name=f"Qsb{d}")
                for d in range(LH)]
        V_sb = [P.tile([128, DL], bf16, tag=f"V{t}", name=f"Vsb{t}")
                for t in range(S // 128)]
        cos_sb = P.tile([128, S], bf16, tag="cos")
        sin_sb = P.tile([128, S], bf16, tag="sin")
        tri_sb = P.tile([128, 128], bf16, tag="tri")
        bq_sb = P.tile([128, LH], f32, tag="bq")
        bk_sb = P.tile([128, LH], f32, tag="bk")
        onec_sb = P.tile([128, 1], bf16, tag="onec")
        oner_sb = P.tile([1, 128], f32r, tag="oner")
        wo_sb = [WOP.tile([128, HID], bf16, tag=f"wo{h}", name=f"wosb{h}")
                 for h in range(LH)]

        def rope(pool, raw, dst, sl):
            """dst = raw*cos + rotate_half(raw)*sin; the rotate is folded
            into partition-offset reads against a half-sign-flipped sin
            table (sin_sb rows >=64 carry the minus sign)."""
            m1 = pool.tile([128, SC], bf16, tag="rm1", bufs=2)
            nc.vector.tensor_mul(m1[:], raw[:], cos_sb[:, sl])
            m2 = pool.tile([128, SC], bf16, tag="rm2", bufs=2)
            nc.vector.tensor_mul(m2[0:64, :], raw[64:128, :],
                                 sin_sb[64:128, sl])
            nc.vector.tensor_mul(m2[64:128, :], raw[0:64, :],
                                 sin_sb[0:64, sl])
            nc.vector.tensor_add(dst, m1[:], m2[:])

        # ---- phase 1: K^T (roped), V, Q^T (roped) for the whole sequence ----
        with tc.tile_pool(name="p1w", bufs=1) as WP, \
             tc.tile_pool(name="p1x", bufs=1) as XP, \
             tc.tile_pool(name="p1t", bufs=2) as TP, \
             tc.tile_pool(name="p1kq", bufs=4, space="PSUM") as PKQ, \
             tc.tile_pool(name="p1v", bufs=4, space="PSUM") as PV:
            wk_sb = [WP.tile([128, DL], bf16, tag=f"wk{h}", name=f"wksb{h}")
                     for h in range(HCH)]
            wv_sb = [WP.tile([128, DL], bf16, tag=f"wv{h}", name=f"wvsb{h}")
                     for h in range(HCH)]
            wq_sb = [WQP.tile([128, DL], bf16, tag=f"wq{h}", name=f"wqsb{h}")
                     for h in range(HCH)]
            # one chunk of x tiles in flight plus the next being fetched
            x_t = [[XP.tile([128, SC], bf16, tag=f"x{h}_{j % 2}",
                            name=f"xt{j}_{h}") for h in range(HCH)]
                   for j in range(NSC)]

            # two HWDGE queues in parallel: weights stream on the SP queue
            # while x chunk 0 streams on the Activation queue, so chunk-0
            # K-projection (one wk+x pair per 864ns) is never DMA-starved.
            for h in range(HCH):
                nc.sync.dma_start(wk_sb[h][:], wkT[128 * h:128 * (h + 1), :])
                nc.sync.dma_start(x_t[0][h][:], xT[128 * h:128 * (h + 1),
                                                   0:SC])
            for h in range(HCH):
                nc.sync.dma_start(wv_sb[h][:], wvT[128 * h:128 * (h + 1), :])
            nc.scalar.dma_start(cos_sb[:], cosT[:])
            nc.scalar.dma_start(sin_sb[:], sinT[:])
            nc.scalar.dma_start(tri_sb[:], trim[:])
            nc.scalar.dma_start(onec_sb[:], onec[:])
            nc.scalar.dma_start(oner_sb[:], oner[:])
            nc.scalar.dma_start(bq_sb[:], bq2[:])
            nc.scalar.dma_start(bk_sb[:], bk2[:])
            # trigger the one-off ACT_TABLE_LOAD while the PE is projecting
            warm = TP.tile([128, 1], f32, tag="warm", bufs=1)
            nc.scalar.activation(warm[:], bk_sb[:, 0:1], Exp)
            for h in range(HCH):
                nc.sync.dma_start(wq_sb[h][:], wqT[128 * h:128 * (h + 1), :])
            for h in range(HCH):
                nc.sync.dma_start(x_t[1][h][:], xT[128 * h:128 * (h + 1),
                                                   SC:2 * SC])
            for h in range(LH):
                nc.sync.dma_start(wo_sb[h][:], woT[128 * h:128 * (h + 1), :])

            for j in range(NSC):
                sl = slice(SC * j, SC * (j + 1))
                if j >= 1:
                    nj = j + 1
                    if nj < NSC:
                        for h in range(HCH):
                            nc.sync.dma_start(
                                x_t[nj][h][:],
                                xT[128 * h:128 * (h + 1),
                                   SC * nj:SC * (nj + 1)])
                # K projection (4 banks), then V (4 banks), then Q reusing
                # K's banks after the Act engine drained them.
                psk = [PKQ.tile([128, SC], f32, tag="pkq",
                                name=f"psk{j}_{d}") for d in range(LH)]
                for h in range(HCH):
                    for d in range(LH):
                        nc.tensor.matmul(psk[d][:],
                                         wk_sb[h][:, 128 * d:128 * (d + 1)],
                                         x_t[j][h][:], start=(h == 0),
                                         stop=(h == HCH - 1))
                psv = [PV.tile([128, DL], f32, tag="pv",
                               name=f"psv{j}_{st}") for st in range(4)]
                for h in range(HCH):
                    for st in range(4):
                        nc.tensor.matmul(psv[st][:],
                                         x_t[j][h][:, 128 * st:128 * (st + 1)],
                                         wv_sb[h][:], start=(h == 0),
                                         stop=(h == HCH - 1))
                for d in range(LH):
                    kraw = TP.tile([128, SC], bf16, tag="kraw", bufs=2)
                    nc.scalar.activation(kraw[:], psk[d][:], Ident,
                                         bias=bk_sb[:, d:d + 1], scale=1.0)
                    rope(TP, kraw, K_sb[d][:, sl], sl)
                psq = [PKQ.tile([128, SC], f32, tag="pkq",
                                name=f"psq{j}_{d}") for d in range(LH)]
                for h in range(HCH):
                    for d in range(LH):
                        nc.tensor.matmul(psq[d][:],
                                         wq_sb[h][:, 128 * d:128 * (d + 1)],
                                         x_t[j][h][:], start=(h == 0),
                                         stop=(h == HCH - 1))
                for st in range(4):
                    nc.scalar.copy(V_sb[4 * j + st][:], psv[st][:])
                for d in range(LH):
                    qraw = TP.tile([128, SC], bf16, tag="qraw", bufs=2)
                    nc.scalar.activation(qraw[:], psq[d][:], Ident,
                                         bias=bq_sb[:, d:d + 1], scale=1.0)
                    rope(TP, qraw, Q_sb[d][:, sl], sl)

        # ---- phase 2: attention (pipelined) + deferred out-projection ----
        # pool creation order fixes bank placement: SH+PC on the psq banks
        # (drained right after phase 1), PS on the psv banks (drained early).
        with tc.tile_pool(name="p2sh", bufs=2, space="PSUM") as SH, \
             tc.tile_pool(name="p2pc", bufs=2, space="PSUM") as PC, \
             tc.tile_pool(name="p2ps", bufs=4, space="PSUM") as PS, \
             tc.tile_pool(name="p2ex", bufs=6) as EX, \
             tc.tile_pool(name="p2rc", bufs=2) as RC, \
             tc.tile_pool(name="p2rb", bufs=2) as RB, \
             tc.tile_pool(name="p2ct", bufs=2) as CT, \
             tc.tile_pool(name="p2ot", bufs=4) as OT:

            def outproj_chain(j, ct, oc, qt, last=False):
                """One 4-matmul out-projection chain; the copy rides on DVE
                (Act is exp-saturated while these interleave attention)."""
                osl = slice(SC * oc, SC * (oc + 1))
                pso = PS.tile([128, SC], f32, tag="ps",
                              name=f"pso{j}_{oc}_{qt}")
                for it in range(LH):
                    nc.tensor.matmul(
                        pso[:], ct[it][:, 128 * qt:128 * (qt + 1)],
                        wo_sb[it][:, osl], start=(it == 0),
                        stop=(it == LH - 1))
                ot = OT.tile([128, SC], bf16, tag="ot",
                             name=f"ot{j}_{oc}_{qt}")
                if last and qt % 2 == 0:
                    nc.scalar.copy(ot[:], pso[:])
                else:
                    nc.vector.tensor_copy(ot[:], pso[:])
                # the final chunk's stores split across both HWDGE
                # queues so the drain after the last matmul halves
                dma_eng = nc.scalar if (last and qt % 2) else nc.sync
                dma_eng.dma_start(
                    out[SC * j + 128 * qt:SC * j + 128 * (qt + 1),
                        osl], ot[:])

            def emit_outproj(j, ct, last=False):
                for oc in range(4):
                    for qt in range(4):
                        outproj_chain(j, ct, oc, qt, last)

            pending = None   # (j, ct) outproj deferred one chunk
            ct = [None] * LH

            def emit_norm_tail(j, h, psc, den):
                """den broadcast + reciprocal + ct mul for head h of chunk j
                (deferred one head so the PE never waits on the den copy)."""
                psb = SH.tile([128, SC], f32, tag="sh", name=f"psb{j}_{h}")
                nc.tensor.matmul(psb[:], oner_sb[:], den[:],
                                 start=True, stop=True)
                rb = RB.tile([128, SC], f32, tag="rb", name=f"rb{j}_{h}")
                nc.vector.reciprocal_approx_fast(out=rb[:], in_=psb[:])
                cth = CT.tile([128, SC], bf16, tag=f"ct{h}",
                              name=f"ct{j}_{h}")
                nc.vector.tensor_mul(cth[:], psc[:], rb[:])
                ct[h] = cth

            for j in range(NSC):
                T = 4 * j + 4
                norm_pend = None
                # interleave the previous chunk's out-projection chains into
                # this chunk's attention tile stream: the PE fills the slack
                # of the (slightly slower) exp pipeline instead of running
                # the out-projection as an Act-idle block afterwards.
                op_j, op_ct = pending if pending is not None else (None, None)
                op_emitted, tiles_done, tiles_total = 0, 0, LH * T

                for h in range(LH):
                    psc = PC.tile([128, SC], f32, tag="pc",
                                  name=f"psc{j}_{h}")
                    psd = SH.tile([1, SC], f32, tag="sh",
                                  name=f"psd{j}_{h}")
                    exs = [None] * T

                    def emit_score(t):
                        p = t - 4 * j  # >=0 for diagonal tiles
                        c0 = 128 * p if p > 0 else 0
                        cs = slice(c0, SC)
                        ps = PS.tile([128, SC], f32, tag="ps",
                                     name=f"pss{j}_{h}_{t}")
                        nc.tensor.matmul(ps[:, cs],
                                         K_sb[h][:, 128 * t:128 * (t + 1)],
                                         Q_sb[h][:, SC * j + c0:SC * (j + 1)],
                                         start=True, stop=True)
                        ex = EX.tile([128, SC], bf16, tag="ex",
                                     name=f"ex{j}_{h}_{t}")
                        nc.scalar.activation(ex[:, cs], ps[:, cs], Exp)
                        if p >= 0:
                            dsl = slice(128 * p, 128 * (p + 1))
                            nc.vector.tensor_mul(ex[:, dsl], ex[:, dsl],
                                                 tri_sb[:])
                        exs[t] = ex

                    for t in range(min(3, T)):
                        emit_score(t)
                    for t in range(T):
                        if t + 3 < T:
                            emit_score(t + 3)
                        p = t - 4 * j
                        cs = slice(128 * p if p > 0 else 0, SC)
                        nc.tensor.matmul(psd[:, cs], onec_sb[:],
                                         exs[t][:, cs],
                                         start=(t == 0), stop=(t == T - 1))
                        nc.tensor.matmul(psc[:, cs],
                                         V_sb[t][:, 128 * h:128 * (h + 1)],
                                         exs[t][:, cs], start=(t == 0),
                                         stop=(t == T - 1))
                        tiles_done += 1
                        while (op_j is not None and op_emitted < 16 and
                               16 * tiles_done >= tiles_total *
                               (op_emitted + 1)):
                            outproj_chain(op_j, op_ct, op_emitted // 4,
                                          op_emitted % 4)
                            op_emitted += 1
                    den = RC.tile([1, SC], f32r, tag="den",
                                  name=f"den{j}_{h}")
                    nc.vector.tensor_copy(den[:], psd[:])
                    if norm_pend is not None:
                        emit_norm_tail(*norm_pend)
                    norm_pend = (j, h, psc, den)
                while op_j is not None and op_emitted < 16:
                    outproj_chain(op_j, op_ct, op_emitted // 4,
                                  op_emitted % 4)
                    op_emitted += 1
                emit_norm_tail(*norm_pend)
                pending = (j, list(ct))
            emit_outproj(*pending, last=True)
    nc.compile()
    return nc


def _get_nc():
    if "nc" not in _CACHE:
        _CACHE["nc"] = _build_nc()
    return _CACHE["nc"]


def _consts():
    if "consts" not in _CACHE:
        inv = (10000.0 ** (-np.arange(0, D, 2, dtype=np.float64) / D))
        t = np.arange(S, dtype=np.float64)
        fr = np.outer(t, inv)                      # [S, 64]
        cos = np.concatenate([np.cos(fr)] * 2, 1).T
        sin = np.concatenate([np.sin(fr)] * 2, 1).T.copy()
        sin[64:] *= -1.0
        tri = (np.arange(128)[:, None] <= np.arange(128)[None, :])
        _CACHE["consts"] = {
            "cosT": np.ascontiguousarray(cos.astype(ml_dtypes.bfloat16)),
            "sinT": np.ascontiguousarray(sin.astype(ml_dtypes.bfloat16)),
            "trim": np.ascontiguousarray(
                tri.astype(ml_dtypes.bfloat16)),
            "onec": np.ones((128, 1), ml_dtypes.bfloat16),
            "oner": np.ones((1, 128), np.float32),
        }
    return _CACHE["consts"]


def _marshal(hidden_states, Wq, bq, Wk, bk, Wv, bv, Wo, bo):
    consts = _consts()
    scale = 1.0 / np.sqrt(D)
    xTs = [np.ascontiguousarray(
        hidden_states[b].T.astype(ml_dtypes.bfloat16)) for b in range(2)]
    in_maps = []
    for c in range(N_CORES):
        b, hg = c // 4, c % 4
        rows = slice(DL * hg, DL * (hg + 1))
        m = dict(consts)
        m["xT"] = xTs[b]
        m["wqT"] = np.ascontiguousarray(
            (Wq[rows] * scale).T.astype(ml_dtypes.bfloat16))
        m["wkT"] = np.ascontiguousarray(Wk[rows].T.astype(ml_dtypes.bfloat16))
        m["wvT"] = np.ascontiguousarray(Wv[rows].T.astype(ml_dtypes.bfloat16))
        m["woT"] = np.ascontiguousarray(
            Wo[:, rows].T.astype(ml_dtypes.bfloat16))
        m["bq2"] = np.ascontiguousarray(
            (bq[rows] * scale).reshape(LH, 128).T.astype(np.float32))
        m["bk2"] = np.ascontiguousarray(
            bk[rows].reshape(LH, 128).T.astype(np.float32))
        in_maps.append(m)
    return in_maps


def _gather(results, bias):
    out = np.empty((2, S, HID), np.float32)
    for b in range(2):
        acc = results[4 * b]["out"].astype(np.float32).copy()
        for g in range(1, 4):
            acc += results[4 * b + g]["out"]
        out[b] = acc + bias
    return out


def _run(inputs, **kw):
    nc = _get_nc()
    in_maps = _marshal(**{k: np.asarray(v) for k, v in inputs.items()})
    return run_bass_kernel_spmd(nc, in_maps, core_ids=list(range(N_CORES)),
                                **kw)


def _host_bias(inputs):
    Wo = np.asarray(inputs["Wo"], np.float64)
    bv = np.asarray(inputs["bv"], np.float64)
    bo = np.asarray(inputs["bo"], np.float64)
    return (bo + Wo @ bv).astype(np.float32)


def kernel(**inputs):
    res = _run(inputs)
    return _gather(res.results, _host_bias(inputs))


def kernel_traced(**inputs):
    """Like kernel() but with NTFF profiling; returns (output, results)."""
    import types

    try:
        import antenv.axon_hooks  # noqa: F401
    except ImportError:
        from trn_agent_boot.trn_boot import _ntff_profile_via_ctypes
        hook = _ntff_profile_via_ctypes("/opt/axon/libaxon_pjrt.so")
        mod = types.ModuleType("antenv.axon_hooks")
        mod.get_axon_ntff_profile_hook = lambda: hook
        mod.set_axon_ntff_profile_hook = lambda h: None
        sys.modules["antenv.axon_hooks"] = mod
    res = _run(inputs, trace=True)
    return _gather(res.results, _host_bias(inputs)), res
